# revision 14
# baseline (speedup 1.0000x reference)
"""NetVLAD forward on 8 Trainium2 NeuronCores — "flipped" fp16 design, r2.

Full inputs: x [16, 128, 64, 64] f32, conv_w [64, 128], conv_b [64],
centroids [64, 128]. Output [16, 8192] f32.

Sharding: data-parallel over batch — 2 samples per core; weights replicated.

Math per sample (C=128 channels, N=4096 positions, K=64 clusters):
  r[n]  = 1/||x[:, n]||          l = (W @ x)*r[n]   (logits, normalized x)
  es    = exp(l)                 S[n] = sum_k es[n,k]*exp(b[k])
  a     = es*exp(b)/S            (true softmax; exp(b) cancels out of each
                                  vlad ROW under intra-normalization, so the
                                  vlad path uses raw es and only S needs b)
  vlad  = sum_n es[n,k]*(r*rs)[n]*x[:,n] - centroids[k]*A[k],  rs = 1/S
  out   = rownorm(vlad) / 8

Key structure per core (BS=2 samples):
  - x DMA-cast to fp16 (gpsimd SWDGE); x^2 mostly on GPSIMD.
  - Per 128-position chunk: x-chunk is the PE STATIONARY (fp16 -> FWL); one
    matmul streams [W_t/sqrt(C) | I] -> psum [lt | xt] (transposed logits
    AND transposed x, no separate transposes). x^2-chunk stationary streams
    a ones column -> ns[n] n-partitioned.
  - NO Ln/Sqrt: every rsqrt via reciprocal-seeded Newton on DVE, so the
    whole kernel loads ONE activation table set (exp only).
  - exp(lt, scale=y) PSUM->SBUF fp16 = es (psum move comes free; the
    1/sqrt(C) of r is pre-folded into W_t).
  - S: one pair tensor_mul (es*expb) + one pair tensor_reduce per chunk.
  - xt copy+comb fold: DVE pair tensor_tensor with a stride-0 broadcast of
    comb, or ACT copy-with-scale (split to balance engines).
  - mm2: stationary es [n, (s,k)], moving [xtc(256) | rs cols(2)] in ONE
    matmul per chunk, accumulated into psum [128, 258].
"""

import contextlib

import numpy as np

import concourse.bass as bass
import concourse.bacc as bacc
import concourse.tile as tile
from concourse import mybir
from concourse.bass_utils import run_bass_kernel_spmd
from concourse.masks import make_identity

f32 = mybir.dt.float32
f16 = mybir.dt.float16
AF = mybir.ActivationFunctionType
ALU = mybir.AluOpType

B, C, N, K = 16, 128, 4096, 64
NCORES = 8
BS = B // NCORES          # samples per core = 2
CH = 128                  # positions per chunk
NCH = N // CH             # 32 chunks per sample
NH = 4                    # ns/Newton quarters
CPH = NCH // NH           # 16 chunks per half
MB = 4                    # rs/comb minibatch (psum residency bound)
RC = float(1.0 / np.sqrt(C))   # 1/sqrt(128), pre-folded into W_t
FINS = 6e-6               # finalize rowns pre-scale (typ. rowns ~ 1.7e5)


def _build():
    nc = bacc.Bacc("TRN2", target_bir_lowering=False, debug=False,
                   num_devices=NCORES)
    x_h = nc.dram_tensor("x", [BS, C, N], f32, kind="ExternalInput")
    w_h = nc.dram_tensor("conv_w", [K, C], f32, kind="ExternalInput")
    b_h = nc.dram_tensor("conv_b", [K], f32, kind="ExternalInput")
    c_h = nc.dram_tensor("centroids", [K, C], f32, kind="ExternalInput")
    o_h = nc.dram_tensor("out", [BS, K * C], f32, kind="ExternalOutput")

    with tile.TileContext(nc) as tc:
        _emit(nc, tc, x_h, w_h, b_h, c_h, o_h)
    nc.compile()
    return nc


def _newton_rsqrt(nc, pool, y, ns_sb, nsslice, pre, csc, steps, name):
    """y[:, nsslice] ~= rsqrt(ns * pre) via recip seed + Newton steps.

    seed: y0 = 1/(0.5 + 0.5*pre*ns)   (|err| <= ~14% for pre*ns in [0.3,3])
    step: y  = y*(1.5 - 0.5*pre*ns*y^2)
    csc = -0.5*pre (the tensor_scalar fold for the step).
    """
    ysl = y[:, nsslice]
    t0 = pool.tile(list(ysl.shape), f32, tag="nt0", name=f"nt0_{name}")
    nc.vector.tensor_scalar(out=t0[:], in0=ns_sb, scalar1=0.5 * pre,
                            scalar2=0.5, op0=ALU.mult, op1=ALU.add)
    nc.vector.reciprocal(out=ysl, in_=t0[:])
    for it in range(steps):
        a = pool.tile(list(ysl.shape), f32, tag="nta", name=f"nta_{name}{it}")
        nc.vector.tensor_mul(out=a[:], in0=ysl, in1=ysl)
        b = pool.tile(list(ysl.shape), f32, tag="ntb", name=f"ntb_{name}{it}")
        nc.vector.tensor_mul(out=b[:], in0=a[:], in1=ns_sb)
        c = pool.tile(list(ysl.shape), f32, tag="ntc", name=f"ntc_{name}{it}")
        nc.vector.tensor_scalar(out=c[:], in0=b[:], scalar1=csc,
                                scalar2=1.5, op0=ALU.mult, op1=ALU.add)
        nc.vector.tensor_mul(out=ysl, in0=ysl, in1=c[:])


def _emit(nc, tc, x_h, w_h, b_h, c_h, o_h):
    ctx = contextlib.ExitStack()
    with ctx:
        const = ctx.enter_context(tc.tile_pool(name="const", bufs=1))
        esp = ctx.enter_context(tc.tile_pool(name="esp", bufs=6))
        xtp = ctx.enter_context(tc.tile_pool(name="xtp", bufs=2))
        sml = ctx.enter_context(tc.tile_pool(name="sml", bufs=3))
        fin = ctx.enter_context(tc.tile_pool(name="fin", bufs=4))
        ps_ab = ctx.enter_context(tc.tile_pool(name="ps_ab", bufs=5,
                                               space="PSUM"))
        ps_ns = ctx.enter_context(tc.tile_pool(name="ps_ns", bufs=2,
                                               space="PSUM"))
        ps_v = ctx.enter_context(tc.tile_pool(name="ps_v", bufs=1,
                                              space="PSUM"))

        # ---- constants first: tiny DMAs / setup ahead of the big loads ----
        # b replicated across partitions (DRAM bcast DMA), then exp(b) fp16.
        # This MUST be at the head of the gpsimd queue: the in-order ACT
        # stream starts with expb, and every exp sits behind it.
        b_ap = b_h[:]
        b_bc = bass.AP(tensor=b_ap.tensor, offset=b_ap.offset,
                       ap=[[0, 128], [1, K]])
        b_rep = const.tile([128, K], f32, tag="b_rep")
        nc.gpsimd.dma_start(out=b_rep[:], in_=b_bc)
        cent = const.tile([K, C], f32, tag="cent")
        nc.sync.dma_start(out=cent[:], in_=c_h[:, :])
        w_sb = sml.tile([K, C], f32, tag="w_sb")
        nc.sync.dma_start(out=w_sb[:], in_=w_h[:, :])
        ident = const.tile([128, 128], f32, tag="ident")
        make_identity(nc, ident[:])
        expb = const.tile([128, K], f16, tag="expb")
        nc.scalar.activation(out=expb[:], in_=b_rep[:], func=AF.Exp)

        # ---- x load: fp16 cast DMA, quarter pieces, sample-interleaved ----
        x_sb = [const.tile([C, N], f16, tag=f"xsb{s}", name=f"xsb{s}")
                for s in range(BS)]
        xsq = [const.tile([C, N], f16, tag=f"xsq{s}", name=f"xsq{s}")
               for s in range(BS)]
        NQ = 4
        for q in range(NQ):
            sl = slice(q * (N // NQ), (q + 1) * (N // NQ))
            for s in range(BS):
                nc.gpsimd.dma_start(out=x_sb[s][:, sl], in_=x_h[s, :, sl])
        # x^2: first half + one q2 piece on DVE (idle during the load)
        for q in range(NQ):
            sl = slice(q * (N // NQ), (q + 1) * (N // NQ))
            for s in range(BS):
                eng = nc.vector if (q < 2 or (q == 2 and s == 0)) \
                    else nc.gpsimd
                eng.tensor_mul(out=xsq[s][:, sl], in0=x_sb[s][:, sl],
                               in1=x_sb[s][:, sl])

        # combined mm1 moving operand [w_t/sqrt(C) | I] in fp16
        rhs_wi = const.tile([C, K + C], f16, tag="rhs_wi")
        ps_wt = ps_ns.tile([128, K], f32, tag="ns", name="ps_wt")
        nc.tensor.transpose(ps_wt[:], w_sb[:], ident[0:K, 0:K])
        nc.vector.tensor_scalar(out=rhs_wi[:, 0:K], in0=ps_wt[:],
                                scalar1=RC, scalar2=None, op0=ALU.mult)
        nc.vector.tensor_copy(out=rhs_wi[:, K:K + C], in_=ident[:])

        ones_col = const.tile([C, 1], f16, tag="ones_col")
        nc.vector.memset(ones_col[:], 1.0)

        # per-(chunk,sample) column tables, col = 2*ci + s
        y_all = const.tile([128, 2 * NCH], f32, tag="y_all")
        s_all = const.tile([128, 2 * NCH], f32, tag="s_all")
        comb_all = const.tile([128, 2 * NCH], f32, tag="comb_all")

        ps_vlad = ps_v.tile([128, 258], f32, tag="vlad")

        # ---- main loop ----
        ab_t = {}
        es_t = {}
        for half in range(NH):
            ns_ps = ps_ns.tile([128, 2 * CPH], f32, tag="ns",
                               name=f"ns_{half}")
            # ns matmuls for the whole half (PE, n-partitioned output)
            for cl in range(CPH):
                ci = half * CPH + cl
                for s in range(BS):
                    nc.tensor.matmul(
                        ns_ps[:, 2 * cl + s:2 * cl + s + 1],
                        xsq[s][:, ci * CH:(ci + 1) * CH],
                        ones_col[:], start=True, stop=True)
            # y = rsqrt(ns/C) on DVE (Newton); exp scale uses y directly
            # since W_t carries the 1/sqrt(C).
            ns_sb = sml.tile([128, 2 * CPH], f32, tag="ns_sb",
                             name=f"ns_sb_{half}")
            nc.vector.tensor_copy(out=ns_sb[:], in_=ns_ps[:])
            hsl = slice(2 * half * CPH, 2 * (half + 1) * CPH)
            _newton_rsqrt(nc, sml, y_all, ns_sb[:], hsl,
                          pre=1.0 / C, csc=-0.5 / C, steps=2,
                          name=f"h{half}")

            for cl in range(CPH):
                ci = half * CPH + cl
                # psum [lt0 | xt0 | lt1 | xt1]
                ab = ps_ab.tile([128, 2 * (K + C)], f32, tag="ab",
                                name=f"ab_{ci}")
                ab_t[ci] = ab
                for s in range(BS):
                    off = s * (K + C)
                    nc.tensor.matmul(
                        ab[:, off:off + K + C],
                        x_sb[s][:, ci * CH:(ci + 1) * CH],
                        rhs_wi[:], start=True, stop=True)

                es = esp.tile([128, 2, K], f16, tag="es", name=f"es_{ci}")
                es_t[ci] = es
                for s in range(BS):
                    off = s * (K + C)
                    col = 2 * ci + s
                    # es = exp(y * lt): PSUM -> SBUF move comes free
                    nc.scalar.activation(
                        out=es[:, s, :], in_=ab[:, off:off + K],
                        func=AF.Exp, scale=y_all[:, col:col + 1])
                # S-path: es2 = es*expb (pair op), S = row-sums (pair op)
                es2 = esp.tile([128, 2, K], f16, tag="es2",
                               name=f"es2_{ci}")
                eb = expb[:]
                ebv = bass.AP(tensor=eb.tensor, offset=eb.offset,
                              ap=[list(eb.ap[0]), [0, 2], [1, K]])
                nc.gpsimd.tensor_mul(out=es2[:], in0=es[:], in1=ebv)
                nc.vector.tensor_reduce(
                    out=s_all[:, 2 * ci:2 * ci + 2], in_=es2[:],
                    axis=mybir.AxisListType.X, op=ALU.add)

                if ci % MB == MB - 1:
                    # minibatch tail: rs = 1/S; comb = (rs/sqrt(C))*y
                    mb0 = ci - (MB - 1)
                    msl = slice(2 * mb0, 2 * (ci + 1))
                    rs_b = sml.tile([128, 2 * MB], f32, tag="rs",
                                    name=f"rs_{ci}")
                    nc.vector.reciprocal(out=rs_b[:], in_=s_all[:, msl])
                    nc.vector.scalar_tensor_tensor(
                        out=comb_all[:, msl], in0=rs_b[:], scalar=RC,
                        in1=y_all[:, msl], op0=ALU.mult, op1=ALU.mult)
                    # one big moving tile for the minibatch: MB x [xtc|rs|pad]
                    CW = 2 * C + 4
                    xtc = xtp.tile([128, MB * CW], f16, tag="xtc",
                                   name=f"xtc_{ci}")
                    # all MB rs column pairs in one strided fp16 copy
                    rsv = xtc[:].rearrange("p (a b) -> p a b", b=CW)
                    nc.vector.tensor_copy(out=rsv[:, :, 2 * C:2 * C + 2],
                                          in_=rs_b[:].rearrange(
                                              "p (a b) -> p a b", b=2))
                    for cj in range(mb0, ci + 1):
                        abj = ab_t.pop(cj)
                        esj = es_t.pop(cj)
                        base = (cj - mb0) * CW
                        xv = abj[:].rearrange("p (a b) -> p a b", b=K + C)
                        xv = xv[:, :, K:K + C]
                        cb = comb_all[:, 2 * cj:2 * cj + 2]
                        cbv = bass.AP(
                            tensor=cb.tensor, offset=cb.offset,
                            ap=[list(cb.ap[0]), [1, 2], [0, C]])
                        nc.vector.tensor_mul(
                            out=xtc[:, base:base + 2 * C].rearrange(
                                "p (a b) -> p a b", a=2),
                            in0=xv, in1=cbv)
                        # mm2: stationary es, moving [xtc(256) | rs(2)]
                        nc.tensor.matmul(
                            ps_vlad[:, 0:2 * C + 2],
                            esj[:].rearrange("p a b -> p (a b)"),
                            xtc[:, base:base + 2 * C + 2],
                            start=(cj == 0), stop=(cj == NCH - 1))

        # ---- finalize: centroid subtract -> rownorm -> /8 -> out ----
        rowns = fin.tile([K, BS], f32, tag="rowns")
        t2s = []
        for s in range(BS):
            vsl = ps_vlad[s * K:(s + 1) * K, s * C:(s + 1) * C]
            a_col = ps_vlad[s * K:(s + 1) * K, 2 * C + s:2 * C + s + 1]
            a_sb = fin.tile([K, 1], f32, tag="a_sb", name=f"a_{s}")
            nc.vector.tensor_copy(out=a_sb[:], in_=a_col)
            t1 = fin.tile([K, C], f32, tag="t1", name=f"t1_{s}")
            nc.vector.tensor_scalar(out=t1[:], in0=cent[:], scalar1=a_sb[:],
                                    scalar2=None, op0=ALU.mult)
            t2 = fin.tile([K, C], f32, tag="t2", name=f"t2_{s}")
            nc.vector.tensor_sub(out=t2[:], in0=vsl, in1=t1[:])
            t2s.append(t2)
            # rowns = sum(t2^2), square fused via scalar_tensor_tensor
            sq2 = fin.tile([K, C], f32, tag="sq2", name=f"sq2_{s}")
            nc.vector.scalar_tensor_tensor(
                out=sq2[:], in0=t2[:], scalar=1.0, in1=t2[:],
                op0=ALU.mult, op1=ALU.mult,
                accum_out=rowns[:, s:s + 1])
        # y_fin = rsqrt(rowns*FINS) for both samples at once
        y_fin = fin.tile([K, BS], f32, tag="y_fin")
        _newton_rsqrt(nc, fin, y_fin[:], rowns[:], slice(0, BS),
                      pre=FINS, csc=-0.5 * FINS, steps=3, name="fin")
        for s in range(BS):
            # 1/(8*sqrt(rowns)) = y_fin * sqrt(FINS) / 8
            o_sb = fin.tile([K, C], f32, tag="osb", name=f"osb_{s}")
            nc.vector.tensor_scalar(out=o_sb[:], in0=t2s[s][:],
                                    scalar1=y_fin[:, s:s + 1],
                                    scalar2=float(np.sqrt(FINS) / 8.0),
                                    op0=ALU.mult, op1=ALU.mult)
            nc.sync.dma_start(
                out=o_h[s, :].rearrange("(k c) -> k c", c=C), in_=o_sb[:])


_NC = None


def kernel(x, conv_w, conv_b, centroids):
    global _NC
    if _NC is None:
        _NC = _build()
    x = np.ascontiguousarray(np.asarray(x, dtype=np.float32)).reshape(B, C, N)
    conv_w = np.asarray(conv_w, dtype=np.float32)
    conv_b = np.asarray(conv_b, dtype=np.float32)
    centroids = np.asarray(centroids, dtype=np.float32)
    in_maps = [{
        "x": x[i * BS:(i + 1) * BS],
        "conv_w": conv_w,
        "conv_b": conv_b,
        "centroids": centroids,
    } for i in range(NCORES)]
    res = run_bass_kernel_spmd(_NC, in_maps, core_ids=list(range(NCORES)))
    return np.concatenate([res.results[i]["out"] for i in range(NCORES)],
                          axis=0)


# revision 16
# speedup vs baseline: 1.2216x; 1.2216x over previous
"""NetVLAD forward on 8 Trainium2 NeuronCores — "flipped" fp16 design.

Full inputs: x [16, 128, 64, 64] f32, conv_w [64, 128], conv_b [64],
centroids [64, 128]. Output [16, 8192] f32.

Sharding: data-parallel over batch — 2 samples per core; weights replicated.

Per-sample math (C=128 channels, N=4096 positions, K=64 clusters):
  r[n]   = 1/||x[:, n]||                    (channel L2 norm)
  l      = (conv_w @ x) * r[n]              (logits on normalized x)
  es     = exp(l) ; es' = es * exp(b)       (softmax numerator)
  S[n]   = sum_k es'[n,k] ; rs = 1/S
  vlad   = sum_n es'[n,k] * (x[:,n] * r[n]*rs[n]) - centroids[k]*A[k]
  out    = rownorm(vlad) / 8                (global norm == 8 exactly)

Structure per core (BS=2 samples):
  - x DMA-cast to fp16 [C=128, N] per sample (gpsimd SWDGE), x^2 split
    GPSIMD/DVE.
  - Per chunk ci (128 positions, 32 per sample): the x-chunk is the PE
    STATIONARY (fp16); one matmul streams [W_t | I] -> psum
    [lt(64) | xt(128)]: transposed logits AND transposed x in one pass.
    A second stationary (x^2 chunk) streams a ones column -> ns[n]
    (channel norms), n-partitioned.
  - r = exp(-0.5*ln(ns)) on ACT: every rsqrt via ln/exp so the whole kernel
    uses ONE activation table family (natural_log_exp_and_others).
  - exp(lt, scale=r) PSUM->SBUF fp16 = es (the psum move comes free).
  - DVE scalar_tensor_tensor: es' = es * expb AND S = row-sum, one op.
  - DVE tensor_tensor pair-copy: xtc = xt_psum * comb (comb = r*rs,
    free-stride-0 broadcast AP), f32 psum -> fp16 sbuf.
  - mm2: stationary es' [n, (s,k)], moving [xtc_s0|xtc_s1] (256) + rs cols
    (2) accumulated into one psum [128, 258] over all 32 chunks.
  - rs/comb at minibatch (4-chunk) granularity so only 5 ab-psum tiles are
    ever live (8-bank budget).
"""

import contextlib

import numpy as np

import concourse.bass as bass
import concourse.bacc as bacc
import concourse.tile as tile
from concourse import mybir
from concourse.bass_utils import run_bass_kernel_spmd
from concourse.masks import make_identity

f32 = mybir.dt.float32
f16 = mybir.dt.float16
AF = mybir.ActivationFunctionType
ALU = mybir.AluOpType
AX = mybir.AxisListType

B, C, N, K = 16, 128, 4096, 64
NCORES = 8
BS = B // NCORES          # samples per core = 2
CH = 128                  # positions per chunk
NCH = N // CH             # 32 chunks per sample
NB = 4                    # r batches (ns -> ln -> exp granularity)
CPB = NCH // NB           # 8 chunks per r-batch
MB = 4                    # rs/comb minibatch (psum residency bound)
LN8 = float(np.log(8.0))


def _build():
    nc = bacc.Bacc("TRN2", target_bir_lowering=False, debug=False,
                   num_devices=NCORES)
    x_h = nc.dram_tensor("x", [BS, C, N], f32, kind="ExternalInput")
    w_h = nc.dram_tensor("conv_w", [K, C], f32, kind="ExternalInput")
    b_h = nc.dram_tensor("conv_b", [K], f32, kind="ExternalInput")
    c_h = nc.dram_tensor("centroids", [K, C], f32, kind="ExternalInput")
    o_h = nc.dram_tensor("out", [BS, K * C], f32, kind="ExternalOutput")

    with tile.TileContext(nc) as tc:
        _emit(nc, tc, x_h, w_h, b_h, c_h, o_h)
    nc.compile()
    return nc


def _emit(nc, tc, x_h, w_h, b_h, c_h, o_h):
    ctx = contextlib.ExitStack()
    with ctx:
        const = ctx.enter_context(tc.tile_pool(name="const", bufs=1))
        esp = ctx.enter_context(tc.tile_pool(name="esp", bufs=6))
        xtp = ctx.enter_context(tc.tile_pool(name="xtp", bufs=4))
        sml = ctx.enter_context(tc.tile_pool(name="sml", bufs=3))
        fin = ctx.enter_context(tc.tile_pool(name="fin", bufs=4))
        ps_ab = ctx.enter_context(tc.tile_pool(name="ps_ab", bufs=5,
                                               space="PSUM"))
        ps_ns = ctx.enter_context(tc.tile_pool(name="ps_ns", bufs=2,
                                               space="PSUM"))
        ps_v = ctx.enter_context(tc.tile_pool(name="ps_v", bufs=1,
                                              space="PSUM"))

        # ---- constants ----
        ident = const.tile([128, 128], f32, tag="ident")
        make_identity(nc, ident[:])

        # combined mm1 moving operand [w_t | I] in fp16
        rhs_wi = const.tile([C, K + C], f16, tag="rhs_wi")
        w_sb = sml.tile([K, C], f32, tag="w_sb")
        nc.sync.dma_start(out=w_sb[:], in_=w_h[:, :])
        ps_wt = ps_ns.tile([128, K], f32, tag="ns", name="ps_wt")
        nc.tensor.transpose(ps_wt[:], w_sb[:], ident[0:K, 0:K])
        nc.vector.tensor_copy(out=rhs_wi[:, 0:K], in_=ps_wt[:])
        nc.vector.tensor_copy(out=rhs_wi[:, K:K + C], in_=ident[:])

        ones_col = const.tile([C, 1], f16, tag="ones_col")
        nc.vector.memset(ones_col[:], 1.0)

        # b replicated across partitions (DRAM bcast DMA), then exp(b) fp16
        b_ap = b_h[:]
        b_bc = bass.AP(tensor=b_ap.tensor, offset=b_ap.offset,
                       ap=[[0, 128], [1, K]])
        b_rep = const.tile([128, K], f32, tag="b_rep")
        nc.gpsimd.dma_start(out=b_rep[:], in_=b_bc)
        expb = const.tile([128, K], f16, tag="expb")
        nc.scalar.activation(out=expb[:], in_=b_rep[:], func=AF.Exp)

        cent = const.tile([K, C], f32, tag="cent")
        nc.sync.dma_start(out=cent[:], in_=c_h[:, :])

        # per-(chunk,sample) column tables, col = 2*ci + s
        r_all = const.tile([128, 2 * NCH], f32, tag="r_all")
        s_all = const.tile([128, 2 * NCH], f32, tag="s_all")
        comb_all = const.tile([128, 2 * NCH], f32, tag="comb_all")
        rs16_all = const.tile([128, 2 * NCH], f16, tag="rs16_all")

        # ---- x load (fp16 cast DMA, quarter pieces, sample-interleaved) ----
        x_sb = [const.tile([C, N], f16, tag=f"xsb{s}", name=f"xsb{s}")
                for s in range(BS)]
        xsq = [const.tile([C, N], f16, tag=f"xsq{s}", name=f"xsq{s}")
               for s in range(BS)]
        NQ = 4
        for q in range(NQ):
            sl = slice(q * (N // NQ), (q + 1) * (N // NQ))
            for s in range(BS):
                nc.gpsimd.dma_start(out=x_sb[s][:, sl], in_=x_h[s, :, sl])
        # x^2: early pieces on GPSIMD (idle engine), late ones on DVE
        for q in range(NQ):
            sl = slice(q * (N // NQ), (q + 1) * (N // NQ))
            for s in range(BS):
                if q < 3:
                    nc.gpsimd.tensor_mul(out=xsq[s][:, sl],
                                         in0=x_sb[s][:, sl],
                                         in1=x_sb[s][:, sl])
                else:
                    nc.vector.tensor_mul(out=xsq[s][:, sl],
                                         in0=x_sb[s][:, sl],
                                         in1=x_sb[s][:, sl])

        ps_vlad = ps_v.tile([128, 258], f32, tag="vlad")

        # ---- main loop ----
        ab_t = {}
        esn_t = {}
        for batch in range(NB):
            ns_ps = ps_ns.tile([128, 2 * CPB], f32, tag="ns",
                               name=f"ns_{batch}")
            # ns matmuls for the whole batch (PE, n-partitioned output)
            for cl in range(CPB):
                ci = batch * CPB + cl
                for s in range(BS):
                    nc.tensor.matmul(
                        ns_ps[:, 2 * cl + s:2 * cl + s + 1],
                        xsq[s][:, ci * CH:(ci + 1) * CH],
                        ones_col[:], start=True, stop=True)
            # r = exp(-0.5 * ln(ns))
            lnt = sml.tile([128, 2 * CPB], f32, tag="lnt",
                           name=f"lnt_{batch}")
            nc.scalar.activation(out=lnt[:], in_=ns_ps[:], func=AF.Ln)
            bsl = slice(2 * batch * CPB, 2 * (batch + 1) * CPB)
            nc.scalar.activation(out=r_all[:, bsl], in_=lnt[:],
                                 func=AF.Exp, scale=-0.5)

            for cl in range(CPB):
                ci = batch * CPB + cl
                # psum [lt0 | xt0 | lt1 | xt1]
                ab = ps_ab.tile([128, 2 * (K + C)], f32, tag="ab",
                                name=f"ab_{ci}")
                ab_t[ci] = ab
                for s in range(BS):
                    off = s * (K + C)
                    nc.tensor.matmul(
                        ab[:, off:off + K + C],
                        x_sb[s][:, ci * CH:(ci + 1) * CH],
                        rhs_wi[:], start=True, stop=True)

                es = esp.tile([128, 2, K], f16, tag="es", name=f"es_{ci}")
                esn = esp.tile([128, 2 * K], f16, tag="esn",
                               name=f"esn_{ci}")
                esn_t[ci] = esn
                for s in range(BS):
                    off = s * (K + C)
                    col = 2 * ci + s
                    # es = exp(r * lt): PSUM -> SBUF move comes free
                    nc.scalar.activation(
                        out=es[:, s, :], in_=ab[:, off:off + K],
                        func=AF.Exp, scale=r_all[:, col:col + 1])
                    # es' = es * expb ; S = row-sum — one fused DVE op
                    nc.vector.scalar_tensor_tensor(
                        out=esn[:, s * K:(s + 1) * K], in0=es[:, s, :],
                        scalar=1.0, in1=expb[:], op0=ALU.mult,
                        op1=ALU.mult, accum_out=s_all[:, col:col + 1])

                if ci % MB == MB - 1:
                    # minibatch tail: rs = 1/S, comb = rs*r, rs -> fp16
                    mb0 = ci - (MB - 1)
                    msl = slice(2 * mb0, 2 * (ci + 1))
                    rs_b = sml.tile([128, 2 * MB], f32, tag="rs",
                                    name=f"rs_{ci}")
                    nc.vector.reciprocal(out=rs_b[:], in_=s_all[:, msl])
                    nc.vector.tensor_mul(out=comb_all[:, msl], in0=rs_b[:],
                                         in1=r_all[:, msl])
                    nc.vector.tensor_copy(out=rs16_all[:, msl], in_=rs_b[:])

                    for cj in range(mb0, ci + 1):
                        abj = ab_t.pop(cj)
                        esnj = esn_t.pop(cj)
                        xtc = xtp.tile([128, 260], f16, tag="xtc",
                                       name=f"xtc_{cj}")
                        xv = abj[:].rearrange("p (a b) -> p a b", b=K + C)
                        xv = xv[:, :, K:K + C]
                        cb = comb_all[:, 2 * cj:2 * cj + 2]
                        if cj % 5 < 2:
                            # ACT path: per-sample copy-with-scale
                            for s in range(BS):
                                nc.scalar.activation(
                                    out=xtc[:, s * C:(s + 1) * C].rearrange(
                                        "p (a b) -> p a b", a=1),
                                    in_=xv[:, s:s + 1, :], func=AF.Copy,
                                    scale=cb[:, s:s + 1])
                        else:
                            cbv = bass.AP(
                                tensor=cb.tensor, offset=cb.offset,
                                ap=[list(cb.ap[0]), [1, 2], [0, C]])
                            nc.vector.tensor_mul(
                                out=xtc[:, 0:2 * C].rearrange(
                                    "p (a b) -> p a b", a=2),
                                in0=xv, in1=cbv)
                        nc.vector.tensor_copy(
                            out=xtc[:, 2 * C:2 * C + 2],
                            in_=rs16_all[:, 2 * cj:2 * cj + 2])
                        # mm2: stationary es', moving [xtc(256) | rs(2)]
                        nc.tensor.matmul(
                            ps_vlad[:, 0:2 * C + 2], esnj[:],
                            xtc[:, 0:2 * C + 2],
                            start=(cj == 0), stop=(cj == NCH - 1))

        # ---- finalize: centroid subtract -> rownorm -> /8 -> out ----
        for s in range(BS):
            vsl = ps_vlad[s * K:(s + 1) * K, s * C:(s + 1) * C]
            a_col = ps_vlad[s * K:(s + 1) * K, 2 * C + s:2 * C + s + 1]
            a_sb = fin.tile([K, 1], f32, tag="a_sb", name=f"a_{s}")
            nc.vector.tensor_copy(out=a_sb[:], in_=a_col)
            t1 = fin.tile([K, C], f32, tag="t1", name=f"t1_{s}")
            nc.vector.tensor_scalar(out=t1[:], in0=cent[:], scalar1=a_sb[:],
                                    scalar2=None, op0=ALU.mult)
            t2 = fin.tile([K, C], f32, tag="t2", name=f"t2_{s}")
            nc.vector.tensor_sub(out=t2[:], in0=vsl, in1=t1[:])
            # rowns = sum(t2^2) fused with the square
            sq2 = fin.tile([K, C], f32, tag="sq2", name=f"sq2_{s}")
            rowns = fin.tile([K, 1], f32, tag="rowns", name=f"rns_{s}")
            nc.vector.scalar_tensor_tensor(
                out=sq2[:], in0=t2[:], scalar=1.0, in1=t2[:],
                op0=ALU.mult, op1=ALU.mult, accum_out=rowns[:])
            # rn = exp(-0.5*ln(rowns)) = 1/sqrt(rowns); /8 folded below
            lnr = fin.tile([K, 1], f32, tag="lnr", name=f"lnr_{s}")
            nc.scalar.activation(out=lnr[:], in_=rowns[:], func=AF.Ln)
            rn = fin.tile([K, 1], f32, tag="rn", name=f"rn_{s}")
            nc.scalar.activation(out=rn[:], in_=lnr[:], func=AF.Exp,
                                 scale=-0.5)
            o_sb = fin.tile([K, C], f32, tag="osb", name=f"osb_{s}")
            nc.vector.tensor_scalar(out=o_sb[:], in0=t2[:], scalar1=rn[:],
                                    scalar2=0.125, op0=ALU.mult,
                                    op1=ALU.mult)
            nc.sync.dma_start(
                out=o_h[s, :].rearrange("(k c) -> k c", c=C), in_=o_sb[:])


_NC = None


def kernel(x, conv_w, conv_b, centroids):
    global _NC
    if _NC is None:
        _NC = _build()
    x = np.ascontiguousarray(np.asarray(x, dtype=np.float32)).reshape(B, C, N)
    conv_w = np.asarray(conv_w, dtype=np.float32)
    conv_b = np.asarray(conv_b, dtype=np.float32)
    centroids = np.asarray(centroids, dtype=np.float32)
    in_maps = [{
        "x": x[i * BS:(i + 1) * BS],
        "conv_w": conv_w,
        "conv_b": conv_b,
        "centroids": centroids,
    } for i in range(NCORES)]
    res = run_bass_kernel_spmd(_NC, in_maps, core_ids=list(range(NCORES)))
    return np.concatenate([res.results[i]["out"] for i in range(NCORES)],
                          axis=0)


# revision 17
# speedup vs baseline: 1.2764x; 1.0449x over previous
"""NetVLAD forward on 8 Trainium2 NeuronCores — "flipped" fp16 design.

Full inputs: x [16, 128, 64, 64] f32, conv_w [64, 128], conv_b [64],
centroids [64, 128]. Output [16, 8192] f32.

Sharding: data-parallel over batch — 2 samples per core; weights replicated.

Per-sample math (C=128 channels, N=4096 positions, K=64 clusters):
  r[n]   = 1/||x[:, n]||                    (channel L2 norm)
  l      = (conv_w @ x) * r[n]              (logits on normalized x)
  es     = exp(l) ; es' = es * exp(b)       (softmax numerator)
  S[n]   = sum_k es'[n,k] ; rs = 1/S
  vlad   = sum_n es'[n,k] * (x[:,n] * r[n]*rs[n]) - centroids[k]*A[k]
  out    = rownorm(vlad) / 8                (global norm == 8 exactly)

Structure per core (BS=2 samples):
  - x DMA-cast to fp16 [C=128, N] per sample (gpsimd SWDGE), x^2 split
    GPSIMD/DVE.
  - Per chunk ci (128 positions, 32 per sample): the x-chunk is the PE
    STATIONARY (fp16); one matmul streams [W_t | I] -> psum
    [lt(64) | xt(128)]: transposed logits AND transposed x in one pass.
    A second stationary (x^2 chunk) streams a ones column -> ns[n]
    (channel norms), n-partitioned.
  - r = exp(-0.5*ln(ns)) on ACT: every rsqrt via ln/exp so the whole kernel
    uses ONE activation table family (natural_log_exp_and_others).
  - exp(lt, scale=r) PSUM->SBUF fp16 = es (the psum move comes free).
  - DVE scalar_tensor_tensor: es' = es * expb AND S = row-sum, one op.
  - DVE tensor_tensor pair-copy: xtc = xt_psum * comb (comb = r*rs,
    free-stride-0 broadcast AP), f32 psum -> fp16 sbuf.
  - mm2: stationary es' [n, (s,k)], moving [xtc_s0|xtc_s1] (256) + rs cols
    (2) accumulated into one psum [128, 258] over all 32 chunks.
  - rs/comb at minibatch (4-chunk) granularity so only 5 ab-psum tiles are
    ever live (8-bank budget).
"""

import contextlib

import numpy as np

import concourse.bass as bass
import concourse.bacc as bacc
import concourse.tile as tile
from concourse import mybir
from concourse.bass_utils import run_bass_kernel_spmd
from concourse.masks import make_identity

f32 = mybir.dt.float32
f16 = mybir.dt.float16
AF = mybir.ActivationFunctionType
ALU = mybir.AluOpType
AX = mybir.AxisListType

B, C, N, K = 16, 128, 4096, 64
NCORES = 8
BS = B // NCORES          # samples per core = 2
CH = 128                  # positions per chunk
NCH = N // CH             # 32 chunks per sample
NB = 4                    # r batches (ns -> ln -> exp granularity)
CPB = NCH // NB           # 8 chunks per r-batch
MB = 4                    # rs/comb minibatch (psum residency bound)
LN8 = float(np.log(8.0))


def _build():
    nc = bacc.Bacc("TRN2", target_bir_lowering=False, debug=False,
                   num_devices=NCORES)
    x_h = nc.dram_tensor("x", [BS, C, N], f32, kind="ExternalInput")
    w_h = nc.dram_tensor("conv_w", [K, C], f32, kind="ExternalInput")
    b_h = nc.dram_tensor("conv_b", [K], f32, kind="ExternalInput")
    c_h = nc.dram_tensor("centroids", [K, C], f32, kind="ExternalInput")
    o_h = nc.dram_tensor("out", [BS, K * C], f32, kind="ExternalOutput")

    with tile.TileContext(nc) as tc:
        _emit(nc, tc, x_h, w_h, b_h, c_h, o_h)
    nc.compile()
    return nc


def _emit(nc, tc, x_h, w_h, b_h, c_h, o_h):
    ctx = contextlib.ExitStack()
    with ctx:
        const = ctx.enter_context(tc.tile_pool(name="const", bufs=1))
        esp = ctx.enter_context(tc.tile_pool(name="esp", bufs=6))
        xtp = ctx.enter_context(tc.tile_pool(name="xtp", bufs=4))
        sml = ctx.enter_context(tc.tile_pool(name="sml", bufs=3))
        fin = ctx.enter_context(tc.tile_pool(name="fin", bufs=4))
        ps_ab = ctx.enter_context(tc.tile_pool(name="ps_ab", bufs=5,
                                               space="PSUM"))
        ps_ns = ctx.enter_context(tc.tile_pool(name="ps_ns", bufs=2,
                                               space="PSUM"))
        ps_v = ctx.enter_context(tc.tile_pool(name="ps_v", bufs=1,
                                              space="PSUM"))

        # ---- constants ----
        ident = const.tile([128, 128], f32, tag="ident")
        make_identity(nc, ident[:])

        # combined mm1 moving operand [w_t | I] in fp16
        rhs_wi = const.tile([C, K + C], f16, tag="rhs_wi")
        w_sb = sml.tile([K, C], f32, tag="w_sb")
        nc.sync.dma_start(out=w_sb[:], in_=w_h[:, :])
        ps_wt = ps_ns.tile([128, K], f32, tag="ns", name="ps_wt")
        nc.tensor.transpose(ps_wt[:], w_sb[:], ident[0:K, 0:K])
        nc.vector.tensor_copy(out=rhs_wi[:, 0:K], in_=ps_wt[:])
        nc.vector.tensor_copy(out=rhs_wi[:, K:K + C], in_=ident[:])

        ones_col = const.tile([C, 1], f16, tag="ones_col")
        nc.vector.memset(ones_col[:], 1.0)

        # b replicated across partitions (DRAM bcast DMA), then exp(b) fp16
        b_ap = b_h[:]
        b_bc = bass.AP(tensor=b_ap.tensor, offset=b_ap.offset,
                       ap=[[0, 128], [1, K]])
        b_rep = const.tile([128, K], f32, tag="b_rep")
        nc.gpsimd.dma_start(out=b_rep[:], in_=b_bc)
        expb = const.tile([128, K], f16, tag="expb")
        nc.scalar.activation(out=expb[:], in_=b_rep[:], func=AF.Exp)

        cent = const.tile([K, C], f32, tag="cent")
        nc.sync.dma_start(out=cent[:], in_=c_h[:, :])

        # per-(chunk,sample) column tables, col = 2*ci + s
        r_all = const.tile([128, 2 * NCH], f32, tag="r_all")
        s_all = const.tile([128, 2 * NCH], f32, tag="s_all")
        comb_all = const.tile([128, 2 * NCH], f32, tag="comb_all")
        rs16_all = const.tile([128, 2 * NCH], f16, tag="rs16_all")

        # ---- x load (fp16 cast DMA, quarter pieces, sample-interleaved) ----
        x_sb = [const.tile([C, N], f16, tag=f"xsb{s}", name=f"xsb{s}")
                for s in range(BS)]
        xsq = [const.tile([C, N], f16, tag=f"xsq{s}", name=f"xsq{s}")
               for s in range(BS)]
        NQ = 4
        for q in range(NQ):
            sl = slice(q * (N // NQ), (q + 1) * (N // NQ))
            for s in range(BS):
                nc.gpsimd.dma_start(out=x_sb[s][:, sl], in_=x_h[s, :, sl])
        # x^2: early pieces on GPSIMD (idle engine), late ones on DVE
        for q in range(NQ):
            sl = slice(q * (N // NQ), (q + 1) * (N // NQ))
            for s in range(BS):
                if q < 3:
                    nc.gpsimd.tensor_mul(out=xsq[s][:, sl],
                                         in0=x_sb[s][:, sl],
                                         in1=x_sb[s][:, sl])
                else:
                    nc.vector.tensor_mul(out=xsq[s][:, sl],
                                         in0=x_sb[s][:, sl],
                                         in1=x_sb[s][:, sl])

        ps_vlad = ps_v.tile([128, 258], f32, tag="vlad")

        # ---- main loop ----
        ab_t = {}
        esn_t = {}
        for batch in range(NB):
            ns_ps = ps_ns.tile([128, 2 * CPB], f32, tag="ns",
                               name=f"ns_{batch}")
            # ns matmuls for the whole batch (PE, n-partitioned output)
            for cl in range(CPB):
                ci = batch * CPB + cl
                for s in range(BS):
                    nc.tensor.matmul(
                        ns_ps[:, 2 * cl + s:2 * cl + s + 1],
                        xsq[s][:, ci * CH:(ci + 1) * CH],
                        ones_col[:], start=True, stop=True)
            # r = exp(-0.5 * ln(ns))
            lnt = sml.tile([128, 2 * CPB], f32, tag="lnt",
                           name=f"lnt_{batch}")
            nc.scalar.activation(out=lnt[:], in_=ns_ps[:], func=AF.Ln)
            bsl = slice(2 * batch * CPB, 2 * (batch + 1) * CPB)
            nc.scalar.activation(out=r_all[:, bsl], in_=lnt[:],
                                 func=AF.Exp, scale=-0.5)

            for cl in range(CPB):
                ci = batch * CPB + cl
                # psum [lt0 | xt0 | lt1 | xt1]
                ab = ps_ab.tile([128, 2 * (K + C)], f32, tag="ab",
                                name=f"ab_{ci}")
                ab_t[ci] = ab
                for s in range(BS):
                    off = s * (K + C)
                    nc.tensor.matmul(
                        ab[:, off:off + K + C],
                        x_sb[s][:, ci * CH:(ci + 1) * CH],
                        rhs_wi[:], start=True, stop=True)

                es = esp.tile([128, 2, K], f16, tag="es", name=f"es_{ci}")
                esn = esp.tile([128, 2 * K], f16, tag="esn",
                               name=f"esn_{ci}")
                esn_t[ci] = esn
                for s in range(BS):
                    off = s * (K + C)
                    col = 2 * ci + s
                    # es = exp(r * lt): PSUM -> SBUF move comes free
                    nc.scalar.activation(
                        out=es[:, s, :], in_=ab[:, off:off + K],
                        func=AF.Exp, scale=r_all[:, col:col + 1])
                    # es' = es * expb ; S = row-sum — one fused DVE op
                    nc.vector.scalar_tensor_tensor(
                        out=esn[:, s * K:(s + 1) * K], in0=es[:, s, :],
                        scalar=1.0, in1=expb[:], op0=ALU.mult,
                        op1=ALU.mult, accum_out=s_all[:, col:col + 1])

                if ci % MB == MB - 1:
                    # minibatch tail: rs = 1/S, comb = rs*r, rs -> fp16
                    mb0 = ci - (MB - 1)
                    msl = slice(2 * mb0, 2 * (ci + 1))
                    rs_b = sml.tile([128, 2 * MB], f32, tag="rs",
                                    name=f"rs_{ci}")
                    nc.vector.reciprocal(out=rs_b[:], in_=s_all[:, msl])
                    nc.vector.tensor_mul(out=comb_all[:, msl], in0=rs_b[:],
                                         in1=r_all[:, msl])
                    nc.vector.tensor_copy(out=rs16_all[:, msl], in_=rs_b[:])

                    for cj in range(mb0, ci + 1):
                        abj = ab_t.pop(cj)
                        esnj = esn_t.pop(cj)
                        xtc = xtp.tile([128, 2, C], f16, tag="xtc",
                                       name=f"xtc_{cj}")
                        # xtc[:, s, :] = xt_psum_s * comb[:, 2cj+s]
                        xv = abj[:].rearrange("p (a b) -> p a b", b=K + C)
                        xv = xv[:, :, K:K + C]
                        cb = comb_all[:, 2 * cj:2 * cj + 2]
                        cbv = bass.AP(tensor=cb.tensor, offset=cb.offset,
                                      ap=[list(cb.ap[0]), [1, 2], [0, C]])
                        nc.vector.tensor_mul(out=xtc[:], in0=xv, in1=cbv)
                        # mm2: stationary es', moving [xtc(256) | rs(2)]
                        nc.tensor.matmul(
                            ps_vlad[:, 0:2 * C], esnj[:],
                            xtc[:].rearrange("p a b -> p (a b)"),
                            start=(cj == 0), stop=(cj == NCH - 1))
                        nc.tensor.matmul(
                            ps_vlad[:, 2 * C:2 * C + 2], esnj[:],
                            rs16_all[:, 2 * cj:2 * cj + 2],
                            start=(cj == 0), stop=(cj == NCH - 1))

        # ---- finalize: centroid subtract -> rownorm -> /8 -> out ----
        for s in range(BS):
            vsl = ps_vlad[s * K:(s + 1) * K, s * C:(s + 1) * C]
            a_col = ps_vlad[s * K:(s + 1) * K, 2 * C + s:2 * C + s + 1]
            a_sb = fin.tile([K, 1], f32, tag="a_sb", name=f"a_{s}")
            nc.vector.tensor_copy(out=a_sb[:], in_=a_col)
            t1 = fin.tile([K, C], f32, tag="t1", name=f"t1_{s}")
            nc.vector.tensor_scalar(out=t1[:], in0=cent[:], scalar1=a_sb[:],
                                    scalar2=None, op0=ALU.mult)
            t2 = fin.tile([K, C], f32, tag="t2", name=f"t2_{s}")
            nc.vector.tensor_sub(out=t2[:], in0=vsl, in1=t1[:])
            # rowns = sum(t2^2) fused with the square
            sq2 = fin.tile([K, C], f32, tag="sq2", name=f"sq2_{s}")
            rowns = fin.tile([K, 1], f32, tag="rowns", name=f"rns_{s}")
            nc.vector.scalar_tensor_tensor(
                out=sq2[:], in0=t2[:], scalar=1.0, in1=t2[:],
                op0=ALU.mult, op1=ALU.mult, accum_out=rowns[:])
            # rn = exp(-0.5*ln(rowns)) = 1/sqrt(rowns); /8 folded below
            lnr = fin.tile([K, 1], f32, tag="lnr", name=f"lnr_{s}")
            nc.scalar.activation(out=lnr[:], in_=rowns[:], func=AF.Ln)
            rn = fin.tile([K, 1], f32, tag="rn", name=f"rn_{s}")
            nc.scalar.activation(out=rn[:], in_=lnr[:], func=AF.Exp,
                                 scale=-0.5)
            o_sb = fin.tile([K, C], f32, tag="osb", name=f"osb_{s}")
            nc.vector.tensor_scalar(out=o_sb[:], in0=t2[:], scalar1=rn[:],
                                    scalar2=0.125, op0=ALU.mult,
                                    op1=ALU.mult)
            nc.sync.dma_start(
                out=o_h[s, :].rearrange("(k c) -> k c", c=C), in_=o_sb[:])


_NC = None


def kernel(x, conv_w, conv_b, centroids):
    global _NC
    if _NC is None:
        _NC = _build()
    x = np.ascontiguousarray(np.asarray(x, dtype=np.float32)).reshape(B, C, N)
    conv_w = np.asarray(conv_w, dtype=np.float32)
    conv_b = np.asarray(conv_b, dtype=np.float32)
    centroids = np.asarray(centroids, dtype=np.float32)
    in_maps = [{
        "x": x[i * BS:(i + 1) * BS],
        "conv_w": conv_w,
        "conv_b": conv_b,
        "centroids": centroids,
    } for i in range(NCORES)]
    res = run_bass_kernel_spmd(_NC, in_maps, core_ids=list(range(NCORES)))
    return np.concatenate([res.results[i]["out"] for i in range(NCORES)],
                          axis=0)
